# revision 2
# baseline (speedup 1.0000x reference)
"""CrossModalityAttention Trainium2 kernel.

Full inputs -> full output; internally shards batch B=8192 across 8 NeuronCores
(pure data parallel). Per core: 1024 samples x K=8 modalities = 8192 tokens of
D=1024.

Device strategy (per core):
  - Host pre-transposes weights to [in,out] (lhsT layout) in bf16, folds
    1/sqrt(128) into Wk/bk, folds bv into the residual bias (attention probs
    sum to 1), passes X transposed (d-major bf16) and XB = x + bo + wo@bv
    (token-major f32 residual), plus a [128,128] prior/mask table for the
    16-samples-per-128-token-group score layout.
  - All matmuls run in bf16 (full PE rate, LDWEIGHTS half cost vs fp32);
    PSUM accumulates f32. Softmax runs q-major: scores S[q,k] land with q on
    partitions, so Z = free-dim reduce (DVE), 1/Z normalize is a
    per-partition broadcast multiply -- no cross-partition ops. Off-diagonal
    sample pairs get -30 from the prior/mask table so exp() kills them.
  - Normalized probs are PE-transposed (bf16, 1 cycle/row) to k-major and
    O^T = V^T @ P^T is produced directly in the layout the output projection
    needs -- no fp32 transposes, no per-head Z matmuls.
  - PSUM->SBUF moves run on DVE (tensor_copy), keeping the ACT engine on a
    single table set (Identity/Exp/Ln) to avoid ACT_TABLE_LOAD thrash.
  - Emission is software-pipelined: scores for group g+1 are enqueued on the
    in-order PE queue before the transpose/attend/out-proj of group g, so the
    PE stays fed while the softmax chain (DVE/ACT) of g completes.
  - rstd = exp(-0.5*ln(var+eps)) keeps every ACT function in one table set.
"""

import math

import numpy as np

import concourse.bacc as bacc
import concourse.bass as bass
import concourse.mybir as mybir
import concourse.tile as tile
from concourse.bass_utils import run_bass_kernel_spmd

N_CORES = 8
B, K, D = 8192, 8, 1024
H, HD = 8, 128
BC = B // N_CORES            # samples per core
T = BC * K                   # tokens per core (8192)
TS = 512                     # tokens per tile
NT = T // TS                 # tiles per core
GROUPS = TS // 128           # 128-token groups per tile
SPG = 128 // K               # samples per group (16)
LN_EPS = 1e-5
NEG = -30.0                  # large-negative mask for cross-sample scores

F32 = mybir.dt.float32
BF16 = mybir.dt.bfloat16

_CACHED = None  # compiled Bacc module, built once per process


def _build():
    nc = bacc.Bacc("TRN2", target_bir_lowering=False, debug=False, num_devices=1)

    xtb_d = nc.dram_tensor("XTB", [D, T], BF16, kind="ExternalInput").ap()
    xb_d = nc.dram_tensor("XB", [T, D], F32, kind="ExternalInput").ap()
    wq_d = nc.dram_tensor("WQT", [D, D], BF16, kind="ExternalInput").ap()
    wk_d = nc.dram_tensor("WKT", [D, D], BF16, kind="ExternalInput").ap()
    wv_d = nc.dram_tensor("WVT", [D, D], BF16, kind="ExternalInput").ap()
    wo_d = nc.dram_tensor("WOT", [D, D], BF16, kind="ExternalInput").ap()
    bqk_d = nc.dram_tensor("BQK", [128, 2 * H], F32, kind="ExternalInput").ap()
    pm_d = nc.dram_tensor("PM", [128, 128], F32, kind="ExternalInput").ap()
    eye_d = nc.dram_tensor("EYE", [128, 128], BF16, kind="ExternalInput").ap()
    out_d = nc.dram_tensor("OUT", [T, D], F32, kind="ExternalOutput").ap()

    xtb_r = xtb_d.rearrange("(c p) t -> p c t", p=128)   # [128, 8, T]

    with tile.TileContext(nc) as tc:
        with (
            tc.tile_pool(name="wpool", bufs=1) as wpool,
            tc.tile_pool(name="consts", bufs=1) as consts,
            tc.tile_pool(name="xtbp", bufs=2) as xtbp,
            tc.tile_pool(name="qkp", bufs=2) as qkp,
            tc.tile_pool(name="vp", bufs=2) as vp,
            tc.tile_pool(name="ptp", bufs=2) as ptp,
            tc.tile_pool(name="pnp", bufs=2) as pnp,
            tc.tile_pool(name="ptsp", bufs=2) as ptsp,
            tc.tile_pool(name="otsp", bufs=2) as otsp,
            tc.tile_pool(name="xbp", bufs=2) as xbp,
            tc.tile_pool(name="smalls", bufs=4) as smalls,
            tc.tile_pool(name="projps", bufs=2, space="PSUM") as projps,
            tc.tile_pool(name="attps", bufs=2, space="PSUM") as attps,
            tc.tile_pool(name="ptTp", bufs=2, space="PSUM") as ptTp,
        ):
            # ---- constants / weights (resident) ----
            wq = wpool.tile([128, 8, D], BF16, tag="w_q")
            nc.sync.dma_start(wq[:], wq_d.rearrange("(c p) m -> p c m", p=128))
            wk = wpool.tile([128, 8, D], BF16, tag="w_k")
            nc.sync.dma_start(wk[:], wk_d.rearrange("(c p) m -> p c m", p=128))
            wv = wpool.tile([128, 8, D], BF16, tag="w_v")
            nc.sync.dma_start(wv[:], wv_d.rearrange("(c p) m -> p c m", p=128))
            wo = wpool.tile([128, 8, D], BF16, tag="w_o")
            nc.sync.dma_start(wo[:], wo_d.rearrange("(c p) m -> p c m", p=128))
            bqk = consts.tile([128, 2 * H], F32)
            nc.sync.dma_start(bqk[:], bqk_d)
            pm = consts.tile([128, 128], F32)
            nc.sync.dma_start(pm[:], pm_d)
            eye = consts.tile([128, 128], BF16)
            nc.sync.dma_start(eye[:], eye_d)
            eps = consts.tile([128, 1], F32)
            nc.vector.memset(eps[:], LN_EPS)

            pending = None  # deferred per-group state for software pipelining

            def finish(st8):
                """Transpose probs, attend, out-proj, residual+LN for a group
                whose scores/softmax chain was already emitted."""
                pn, v, g, tok0 = st8
                ptT = ptTp.tile([128, H, 128], BF16)
                for h in range(H):
                    nc.tensor.transpose(ptT[:, h, :], pn[:, h, :], eye[:])
                pts = ptsp.tile([128, H, 128], BF16)
                nc.vector.tensor_copy(pts[:], ptT[:])
                ot = attps.tile([128, H, 128], F32, tag="attps")
                for h in range(H):
                    nc.tensor.matmul(ot[:, h, :], v[:, g, h, :], pts[:, h, :])
                ots = otsp.tile([128, H, 128], BF16)
                nc.vector.tensor_copy(ots[:], ot[:])

                xb = xbp.tile([128, D], F32)
                nc.sync.dma_start(
                    xb[:], xb_d[tok0 + g * 128 : tok0 + (g + 1) * 128, :]
                )
                for half in range(2):
                    yp = projps.tile([128, 512], F32, tag="projps")
                    for c in range(8):
                        nc.tensor.matmul(
                            yp[:],
                            ots[:, c, :],
                            wo[:, c, half * 512 : (half + 1) * 512],
                            start=(c == 0),
                            stop=(c == 7),
                        )
                    nc.vector.tensor_tensor(
                        xb[:, half * 512 : (half + 1) * 512],
                        xb[:, half * 512 : (half + 1) * 512],
                        yp[:],
                        mybir.AluOpType.add,
                    )
                stats = smalls.tile([128, 2, 6], F32, tag="stats")
                for sg in range(2):
                    nc.vector.bn_stats(
                        stats[:, sg, :], xb[:, sg * 512 : (sg + 1) * 512]
                    )
                mv = smalls.tile([128, 2], F32, tag="mv")
                nc.vector.bn_aggr(mv[:], stats[:])
                # rstd = exp(-0.5*ln(var+eps)); ln+exp live in one ACT
                # table set (sqrt does not), avoiding table reloads
                sd = smalls.tile([128, 1], F32, tag="sd")
                nc.scalar.activation(
                    sd[:],
                    mv[:, 1:2],
                    mybir.ActivationFunctionType.Ln,
                    bias=eps[:],
                )
                nc.scalar.activation(
                    sd[:], sd[:], mybir.ActivationFunctionType.Exp, scale=-0.5
                )
                nc.vector.tensor_scalar(
                    out=xb[:],
                    in0=xb[:],
                    scalar1=mv[:, 0:1],
                    scalar2=sd[:],
                    op0=mybir.AluOpType.subtract,
                    op1=mybir.AluOpType.mult,
                )
                nc.sync.dma_start(
                    out_d[tok0 + g * 128 : tok0 + (g + 1) * 128, :], xb[:]
                )

            for t in range(NT):
                tok0 = t * TS
                xtb = xtbp.tile([128, 8, TS], BF16)
                nc.sync.dma_start(xtb[:], xtb_r[:, :, tok0 : tok0 + TS])

                # ---- Q^T, K^T projections (bf16): [d_head(128) x tok(TS)]
                qt = qkp.tile([128, H, TS], BF16, tag="qt")
                kt = qkp.tile([128, H, TS], BF16, tag="kt")
                for wt, dst, bias_col0 in ((wq, qt, 0), (wk, kt, H)):
                    for h in range(H):
                        ps = projps.tile([128, TS], F32, tag="projps")
                        for c in range(8):
                            nc.tensor.matmul(
                                ps[:],
                                wt[:, c, h * HD : (h + 1) * HD],
                                xtb[:, c, :],
                                start=(c == 0),
                                stop=(c == 7),
                            )
                        nc.scalar.activation(
                            dst[:, h, :],
                            ps[:],
                            mybir.ActivationFunctionType.Identity,
                            bias=bqk[:, bias_col0 + h : bias_col0 + h + 1],
                        )

                # ---- V projection (bf16), token-major
                v = vp.tile([128, GROUPS, H, HD], BF16, tag="v")
                for sub in range(GROUPS):
                    for half in range(2):
                        psv = projps.tile([128, 512], F32, tag="projps")
                        for c in range(8):
                            nc.tensor.matmul(
                                psv[:],
                                xtb[:, c, sub * 128 : (sub + 1) * 128],
                                wv[:, c, half * 512 : (half + 1) * 512],
                                start=(c == 0),
                                stop=(c == 7),
                            )
                        nc.vector.tensor_copy(
                            v[:, sub, 4 * half : 4 * half + 4, :],
                            psv.rearrange("p (a b) -> p a b", a=4),
                        )

                # ---- attention per 128-token group, q-major softmax.
                # Scores for group g are emitted, then the deferred
                # transpose/attend/out-proj of the previous group, so the PE
                # queue stays busy while g's softmax chain runs on DVE/ACT.
                for g in range(GROUPS):
                    gsl = slice(g * 128, (g + 1) * 128)
                    st = attps.tile([128, H, 128], F32, tag="attps")
                    for h in range(H):
                        # S[q, k] = sum_d Q^T[d, q] K^T[d, k]
                        nc.tensor.matmul(st[:, h, :], qt[:, h, gsl], kt[:, h, gsl])
                    # add prior/mask (same [128,128] table per head), in place
                    nc.vector.tensor_tensor(
                        st[:],
                        st[:],
                        pm[:, None, :].to_broadcast((128, H, 128)),
                        mybir.AluOpType.add,
                    )
                    pt = ptp.tile([128, H, 128], BF16)
                    nc.scalar.activation(
                        pt[:], st[:], mybir.ActivationFunctionType.Exp
                    )
                    z = smalls.tile([128, H], F32, tag="z")
                    nc.vector.tensor_reduce(
                        z[:], pt[:], mybir.AxisListType.X, mybir.AluOpType.add
                    )
                    rz = smalls.tile([128, H], F32, tag="rz")
                    nc.vector.reciprocal(rz[:], z[:])
                    pn = pnp.tile([128, H, 128], BF16)
                    nc.vector.tensor_tensor(
                        pn[:],
                        pt[:],
                        rz[:, :, None].to_broadcast((128, H, 128)),
                        mybir.AluOpType.mult,
                    )
                    if pending is not None:
                        finish(pending)
                    pending = (pn, v, g, tok0)

            finish(pending)

    nc.compile()
    return nc


def _get_nc():
    global _CACHED
    if _CACHED is None:
        _CACHED = _build()
    return _CACHED


def _reference_numpy(modality_encodings, selection_mask, wq, bq, wk, bk, wv, bv,
                     wo, bo, rel_prior, ln_gamma, ln_beta):
    """Slow fallback, exact port of the reference (used only if inputs fall
    outside the fast path's assumptions: non-trivial mask)."""
    x = modality_encodings.astype(np.float32)
    Bn, Kn, Dn = x.shape
    Hd = Dn // H
    q = (x @ wq.T + bq).reshape(Bn, Kn, H, Hd).transpose(0, 2, 1, 3)
    k = (x @ wk.T + bk).reshape(Bn, Kn, H, Hd).transpose(0, 2, 1, 3)
    v = (x @ wv.T + bv).reshape(Bn, Kn, H, Hd).transpose(0, 2, 1, 3)
    scores = np.einsum("bhqd,bhkd->bhqk", q, k) / math.sqrt(Hd)
    scores = scores + rel_prior[None, None]
    mask2d = (selection_mask[:, :, None] * selection_mask[:, None, :]) > 0
    scores = np.where(mask2d[:, None], scores, -np.inf)
    scores = scores - scores.max(axis=-1, keepdims=True)
    e = np.exp(scores)
    attn = e / e.sum(axis=-1, keepdims=True)
    out = np.einsum("bhqk,bhkd->bhqd", attn, v)
    out = out.transpose(0, 2, 1, 3).reshape(Bn, Kn, Dn)
    out = out @ wo.T + bo
    res = x + out
    mu = res.mean(-1, keepdims=True)
    var = ((res - mu) ** 2).mean(-1, keepdims=True)
    return (res - mu) / np.sqrt(var + LN_EPS) * ln_gamma + ln_beta


def _prep_in_maps(modality_encodings, wq, bq, wk, bk, wv, bv, wo, bo, rel_prior):
    import ml_dtypes

    s = 1.0 / math.sqrt(HD)
    wqt = np.ascontiguousarray(wq.T).astype(ml_dtypes.bfloat16)
    wkt = np.ascontiguousarray((wk * s).T).astype(ml_dtypes.bfloat16)
    wvt = np.ascontiguousarray(wv.T).astype(ml_dtypes.bfloat16)
    wot = np.ascontiguousarray(wo.T).astype(ml_dtypes.bfloat16)
    bks = bk * s
    b_eff = (bo + wo @ bv).astype(np.float32)

    bqk = np.concatenate(
        [bq.reshape(H, HD).T, bks.reshape(H, HD).T], axis=1
    ).astype(np.float32)  # [128, 16]

    # q-major prior/mask table: pmat[q_local, k_local]
    pmat = np.full((128, 128), NEG, dtype=np.float32)
    for sm in range(SPG):
        pmat[sm * K : (sm + 1) * K, sm * K : (sm + 1) * K] = rel_prior
    eye = np.eye(128, dtype=ml_dtypes.bfloat16)

    x_flat = modality_encodings.reshape(B * K, D)
    in_maps = []
    for c in range(N_CORES):
        x_core = x_flat[c * T : (c + 1) * T]
        xt = np.ascontiguousarray(x_core.T)
        in_maps.append({
            "XTB": xt.astype(ml_dtypes.bfloat16),
            "XB": x_core + b_eff,
            "WQT": wqt, "WKT": wkt, "WVT": wvt, "WOT": wot,
            "BQK": bqk, "PM": pmat, "EYE": eye,
        })
    return in_maps


def run_device(inputs, trace=False):
    """Build in_maps from full inputs, run on 8 cores, return (full_out, results)."""
    in_maps = _prep_in_maps(
        inputs["modality_encodings"], inputs["wq"], inputs["bq"], inputs["wk"],
        inputs["bk"], inputs["wv"], inputs["bv"], inputs["wo"], inputs["bo"],
        inputs["rel_prior"],
    )
    nc = _get_nc()
    res = run_bass_kernel_spmd(nc, in_maps, core_ids=list(range(N_CORES)), trace=trace)
    out = np.concatenate(
        [res.results[c]["OUT"].reshape(BC, K, D) for c in range(N_CORES)], axis=0
    )
    return out, res


def kernel(**inputs) -> np.ndarray:
    inputs = {k: np.asarray(v) for k, v in inputs.items()}
    mask = inputs["selection_mask"]
    gamma = inputs["ln_gamma"]
    beta = inputs["ln_beta"]
    if not np.all(mask > 0):
        # general-mask fallback (never hit for the spec'd inputs: fill=ones)
        return _reference_numpy(**{k: inputs[k].astype(np.float32) for k in (
            "modality_encodings", "selection_mask", "wq", "bq", "wk", "bk",
            "wv", "bv", "wo", "bo", "rel_prior", "ln_gamma", "ln_beta")}
        ).astype(np.float32)

    out, _ = run_device(inputs, trace=False)
    # device kernel skips the (identity for spec'd inputs) LN affine params
    if not (np.all(gamma == 1.0) and np.all(beta == 0.0)):
        out = out * gamma + beta
    return out.astype(np.float32)


# revision 10
# speedup vs baseline: 1.1746x; 1.1746x over previous
"""CrossModalityAttention Trainium2 kernel.

Full inputs -> full output; internally shards batch B=8192 across 8 NeuronCores
(pure data parallel). Per core: 1024 samples x K=8 modalities = 8192 tokens of
D=1024.

Device strategy (per core):
  - Host pre-transposes weights to [in,out] (lhsT layout) in bf16, folds
    1/sqrt(128) into Wk/bk, folds bv into the residual bias (attention probs
    sum to 1), passes X transposed (d-major bf16) and XB = x + bo + wo@bv
    (token-major f32 residual), plus a [128,128] prior/mask table for the
    16-samples-per-128-token-group score layout.
  - All matmuls run in bf16 (full PE rate, LDWEIGHTS half cost vs fp32);
    PSUM accumulates f32. Softmax runs q-major: scores S[q,k] land with q on
    partitions, so Z = free-dim reduce (DVE), 1/Z normalize is a
    per-partition broadcast multiply -- no cross-partition ops. Off-diagonal
    sample pairs get -30 from the prior/mask table so exp() kills them.
  - Normalized probs are PE-transposed (bf16, 1 cycle/row) to k-major and
    O^T = V^T @ P^T is produced directly in the layout the output projection
    needs -- no fp32 transposes, no per-head Z matmuls.
  - PSUM->SBUF moves run on DVE (tensor_copy), keeping the ACT engine on a
    single table set (Identity/Exp/Ln) to avoid ACT_TABLE_LOAD thrash.
  - Emission is software-pipelined: scores for group g+1 are enqueued on the
    in-order PE queue before the transpose/attend/out-proj of group g, so the
    PE stays fed while the softmax chain (DVE/ACT) of g completes.
  - rstd = exp(-0.5*ln(var+eps)) keeps every ACT function in one table set.
"""

import math

import numpy as np

import concourse.bacc as bacc
import concourse.bass as bass
import concourse.mybir as mybir
import concourse.tile as tile
from concourse.bass_utils import run_bass_kernel_spmd

N_CORES = 8
B, K, D = 8192, 8, 1024
H, HD = 8, 128
BC = B // N_CORES            # samples per core
T = BC * K                   # tokens per core (8192)
TS = 512                     # tokens per tile
NT = T // TS                 # tiles per core
GROUPS = TS // 128           # 128-token groups per tile
SPG = 128 // K               # samples per group (16)
LN_EPS = 1e-5
NEG = -30.0                  # large-negative mask for cross-sample scores

F32 = mybir.dt.float32
BF16 = mybir.dt.bfloat16

_CACHED = None  # compiled Bacc module, built once per process


def _use_single_act_table():
    """Compile with a single ACT table set: natural_log_exp_and_others holds
    exp+ln+identity+copy together, so the ACT engine never reloads tables
    between Exp (softmax), Identity (bias), Copy (PSUM moves) and Ln (LN).
    The stock act_info.json ordering makes bass pick exp_and_others for Exp
    and natural_log for Ln, which thrashes a 1.3us ACT_TABLE_LOAD twice per
    group. Both the bass-side set chooser and the walrus-side act root are
    pointed at a one-set act_info so their ids agree (id 0)."""
    import functools
    import json
    import os
    import tempfile

    from neuronxcc.driver.Job import Job
    from neuronxcc.driver.jobs.support.FindActInfo import findActInfoFile

    SET = "natural_log_exp_and_others"
    src = findActInfoFile(Job.getPackageDir(), "gen3")
    with open(src) as f:
        d = json.load(f)
    keep = [s for s in d["act_func_sets"] if s["name"] == SET]
    assert keep, f"{SET} missing from act_info.json"
    tmpdir = tempfile.mkdtemp(prefix="act_one_table_")
    srcdir = os.path.dirname(src)
    for s in keep:
        for k in d["pwp_file_keys"]:
            dst = os.path.join(tmpdir, s[k])
            if not os.path.exists(dst):
                os.symlink(os.path.join(srcdir, s[k]), dst)
    out = os.path.join(tmpdir, "act_info.json")
    with open(out, "w") as f:
        json.dump({"pwp_file_keys": d["pwp_file_keys"], "act_func_sets": keep}, f)
    os.environ["BASS_ACT_ROOT_JSON_PATH"] = out

    import concourse.hw_specs as hw_specs

    orig = hw_specs.get_activation_tables
    if getattr(orig, "_single_act_table", False):
        return

    @functools.cache
    def one_table(arch):
        full = orig(arch)
        return {SET: full[SET]}

    one_table._single_act_table = True
    hw_specs.get_activation_tables = one_table
    bacc.get_activation_tables = one_table
    try:
        import concourse.bass_interp as bass_interp

        bass_interp.get_activation_tables = one_table
    except ImportError:
        pass


def _build():
    _use_single_act_table()
    nc = bacc.Bacc("TRN2", target_bir_lowering=False, debug=False, num_devices=1)

    xtb_d = nc.dram_tensor("XTB", [D, T], BF16, kind="ExternalInput").ap()
    xb_d = nc.dram_tensor("XB", [T, D], F32, kind="ExternalInput").ap()
    wq_d = nc.dram_tensor("WQT", [D, D], BF16, kind="ExternalInput").ap()
    wk_d = nc.dram_tensor("WKT", [D, D], BF16, kind="ExternalInput").ap()
    wv_d = nc.dram_tensor("WVT", [D, D], BF16, kind="ExternalInput").ap()
    wo_d = nc.dram_tensor("WOT", [D, D], BF16, kind="ExternalInput").ap()
    bqk_d = nc.dram_tensor("BQK", [128, 2 * H], F32, kind="ExternalInput").ap()
    pm_d = nc.dram_tensor("PM", [128, H * 128], BF16, kind="ExternalInput").ap()
    eye_d = nc.dram_tensor("EYE", [128, 128], BF16, kind="ExternalInput").ap()
    out_d = nc.dram_tensor("OUT", [T, D], F32, kind="ExternalOutput").ap()

    xtb_r = xtb_d.rearrange("(c p) t -> p c t", p=128)   # [128, 8, T]

    with tile.TileContext(nc) as tc:
        with (
            tc.tile_pool(name="wpool", bufs=1) as wpool,
            tc.tile_pool(name="consts", bufs=1) as consts,
            tc.tile_pool(name="xtbp", bufs=2) as xtbp,
            tc.tile_pool(name="qkp", bufs=2) as qkp,
            tc.tile_pool(name="vp", bufs=2) as vp,
            tc.tile_pool(name="ptp", bufs=2) as ptp,
            tc.tile_pool(name="pnp", bufs=2) as pnp,
            tc.tile_pool(name="ptsp", bufs=2) as ptsp,
            tc.tile_pool(name="otsp", bufs=2) as otsp,
            tc.tile_pool(name="xbp", bufs=2) as xbp,
            tc.tile_pool(name="smalls", bufs=4) as smalls,
            tc.tile_pool(name="projps", bufs=2, space="PSUM") as projps,
            tc.tile_pool(name="attps", bufs=2, space="PSUM") as attps,
            tc.tile_pool(name="ptTp", bufs=2, space="PSUM") as ptTp,
        ):
            # ---- constants / weights (resident) ----
            wq = wpool.tile([128, 8, D], BF16, tag="w_q")
            nc.sync.dma_start(wq[:], wq_d.rearrange("(c p) m -> p c m", p=128))
            wk = wpool.tile([128, 8, D], BF16, tag="w_k")
            nc.sync.dma_start(wk[:], wk_d.rearrange("(c p) m -> p c m", p=128))
            wv = wpool.tile([128, 8, D], BF16, tag="w_v")
            nc.sync.dma_start(wv[:], wv_d.rearrange("(c p) m -> p c m", p=128))
            wo = wpool.tile([128, 8, D], BF16, tag="w_o")
            nc.sync.dma_start(wo[:], wo_d.rearrange("(c p) m -> p c m", p=128))
            bqk = consts.tile([128, 2 * H], F32)
            nc.sync.dma_start(bqk[:], bqk_d)
            pm = consts.tile([128, H * 128], BF16)
            nc.sync.dma_start(pm[:], pm_d)
            eye = consts.tile([128, 128], BF16)
            nc.sync.dma_start(eye[:], eye_d)
            eps = consts.tile([128, 1], F32)
            nc.vector.memset(eps[:], LN_EPS)

            pending = None  # deferred per-group state for software pipelining

            def finish(st8):
                """Transpose probs, attend, out-proj, residual+LN for a group
                whose scores/softmax chain was already emitted."""
                pn, v, g, tok0 = st8
                ptT = ptTp.tile([128, H, 128], BF16)
                for h in range(H):
                    nc.tensor.transpose(ptT[:, h, :], pn[:, h, :], eye[:])
                pts = ptsp.tile([128, H, 128], BF16)
                nc.scalar.activation(
                    pts[:], ptT[:], mybir.ActivationFunctionType.Copy
                )
                ot = attps.tile([128, H, 128], F32, tag="attps")
                for h in range(H):
                    nc.tensor.matmul(ot[:, h, :], v[:, g, h, :], pts[:, h, :])
                ots = otsp.tile([128, H, 128], BF16)
                nc.scalar.activation(
                    ots[:], ot[:], mybir.ActivationFunctionType.Copy
                )

                xb = xbp.tile([128, D], F32)
                nc.sync.dma_start(
                    xb[:], xb_d[tok0 + g * 128 : tok0 + (g + 1) * 128, :]
                )
                for half in range(2):
                    yp = projps.tile([128, 512], F32, tag="projps")
                    for c in range(8):
                        nc.tensor.matmul(
                            yp[:],
                            ots[:, c, :],
                            wo[:, c, half * 512 : (half + 1) * 512],
                            start=(c == 0),
                            stop=(c == 7),
                        )
                    nc.vector.tensor_tensor(
                        xb[:, half * 512 : (half + 1) * 512],
                        xb[:, half * 512 : (half + 1) * 512],
                        yp[:],
                        mybir.AluOpType.add,
                    )
                stats = smalls.tile([128, 2, 6], F32, tag="stats")
                for sg in range(2):
                    nc.vector.bn_stats(
                        stats[:, sg, :], xb[:, sg * 512 : (sg + 1) * 512]
                    )
                mv = smalls.tile([128, 2], F32, tag="mv")
                nc.vector.bn_aggr(mv[:], stats[:])
                # rstd = exp(-0.5*ln(var+eps)); ln+exp live in one ACT
                # table set (sqrt does not), avoiding table reloads
                sd = smalls.tile([128, 1], F32, tag="sd")
                nc.scalar.activation(
                    sd[:],
                    mv[:, 1:2],
                    mybir.ActivationFunctionType.Ln,
                    bias=eps[:],
                )
                nc.scalar.activation(
                    sd[:], sd[:], mybir.ActivationFunctionType.Exp, scale=-0.5
                )
                nc.vector.tensor_scalar(
                    out=xb[:],
                    in0=xb[:],
                    scalar1=mv[:, 0:1],
                    scalar2=sd[:],
                    op0=mybir.AluOpType.subtract,
                    op1=mybir.AluOpType.mult,
                )
                nc.sync.dma_start(
                    out_d[tok0 + g * 128 : tok0 + (g + 1) * 128, :], xb[:]
                )

            for t in range(NT):
                tok0 = t * TS
                xtb = xtbp.tile([128, 8, TS], BF16)
                nc.sync.dma_start(xtb[:], xtb_r[:, :, tok0 : tok0 + TS])

                # ---- Q^T, K^T projections (bf16): [d_head(128) x tok(TS)]
                qt = qkp.tile([128, H, TS], BF16, tag="qt")
                kt = qkp.tile([128, H, TS], BF16, tag="kt")
                for wt, dst, bias_col0 in ((wq, qt, 0), (wk, kt, H)):
                    for h in range(H):
                        ps = projps.tile([128, TS], F32, tag="projps")
                        for c in range(8):
                            nc.tensor.matmul(
                                ps[:],
                                wt[:, c, h * HD : (h + 1) * HD],
                                xtb[:, c, :],
                                start=(c == 0),
                                stop=(c == 7),
                            )
                        nc.scalar.activation(
                            dst[:, h, :],
                            ps[:],
                            mybir.ActivationFunctionType.Identity,
                            bias=bqk[:, bias_col0 + h : bias_col0 + h + 1],
                        )

                # ---- V projection (bf16), token-major
                v = vp.tile([128, GROUPS, H, HD], BF16, tag="v")
                for sub in range(GROUPS):
                    for half in range(2):
                        psv = projps.tile([128, 512], F32, tag="projps")
                        for c in range(8):
                            nc.tensor.matmul(
                                psv[:],
                                xtb[:, c, sub * 128 : (sub + 1) * 128],
                                wv[:, c, half * 512 : (half + 1) * 512],
                                start=(c == 0),
                                stop=(c == 7),
                            )
                        nc.vector.tensor_copy(
                            v[:, sub, 4 * half : 4 * half + 4, :],
                            psv.rearrange("p (a b) -> p a b", a=4),
                        )

                # ---- attention per 128-token group, q-major softmax.
                # Scores for group g are emitted, then the deferred
                # transpose/attend/out-proj of the previous group, so the PE
                # queue stays busy while g's softmax chain runs on DVE/ACT.
                for g in range(GROUPS):
                    gsl = slice(g * 128, (g + 1) * 128)
                    st = attps.tile([128, H, 128], F32, tag="attps")
                    # prior/mask lands first via PSUM accumulation:
                    # st[q, (h,k)] = eye^T @ pm, then += Q^T K per head
                    # (two matmuls: a [128,1024] f32 dst would span 2 banks)
                    for hh in range(2):
                        nc.tensor.matmul(
                            st[:, 4 * hh : 4 * hh + 4, :].rearrange(
                                "p a b -> p (a b)"
                            ),
                            eye[:],
                            pm[:, 4 * hh * 128 : (4 * hh + 4) * 128],
                            start=True,
                            stop=False,
                            skip_group_check=True,
                        )
                    for h in range(H):
                        # S[q, k] += sum_d Q^T[d, q] K^T[d, k]
                        nc.tensor.matmul(
                            st[:, h, :],
                            qt[:, h, gsl],
                            kt[:, h, gsl],
                            start=False,
                            stop=True,
                            skip_group_check=True,
                        )
                    pt = ptp.tile([128, H, 128], BF16)
                    nc.scalar.activation(
                        pt[:], st[:], mybir.ActivationFunctionType.Exp
                    )
                    z = smalls.tile([128, H], F32, tag="z")
                    nc.vector.tensor_reduce(
                        z[:], pt[:], mybir.AxisListType.X, mybir.AluOpType.add
                    )
                    rz = smalls.tile([128, H], F32, tag="rz")
                    nc.vector.reciprocal(rz[:], z[:])
                    pn = pnp.tile([128, H, 128], BF16)
                    nc.vector.tensor_tensor(
                        pn[:],
                        pt[:],
                        rz[:, :, None].to_broadcast((128, H, 128)),
                        mybir.AluOpType.mult,
                    )
                    if pending is not None:
                        finish(pending)
                    pending = (pn, v, g, tok0)

            finish(pending)

    nc.compile()
    return nc


def _get_nc():
    global _CACHED
    if _CACHED is None:
        _CACHED = _build()
    return _CACHED


def _reference_numpy(modality_encodings, selection_mask, wq, bq, wk, bk, wv, bv,
                     wo, bo, rel_prior, ln_gamma, ln_beta):
    """Slow fallback, exact port of the reference (used only if inputs fall
    outside the fast path's assumptions: non-trivial mask)."""
    x = modality_encodings.astype(np.float32)
    Bn, Kn, Dn = x.shape
    Hd = Dn // H
    q = (x @ wq.T + bq).reshape(Bn, Kn, H, Hd).transpose(0, 2, 1, 3)
    k = (x @ wk.T + bk).reshape(Bn, Kn, H, Hd).transpose(0, 2, 1, 3)
    v = (x @ wv.T + bv).reshape(Bn, Kn, H, Hd).transpose(0, 2, 1, 3)
    scores = np.einsum("bhqd,bhkd->bhqk", q, k) / math.sqrt(Hd)
    scores = scores + rel_prior[None, None]
    mask2d = (selection_mask[:, :, None] * selection_mask[:, None, :]) > 0
    scores = np.where(mask2d[:, None], scores, -np.inf)
    scores = scores - scores.max(axis=-1, keepdims=True)
    e = np.exp(scores)
    attn = e / e.sum(axis=-1, keepdims=True)
    out = np.einsum("bhqk,bhkd->bhqd", attn, v)
    out = out.transpose(0, 2, 1, 3).reshape(Bn, Kn, Dn)
    out = out @ wo.T + bo
    res = x + out
    mu = res.mean(-1, keepdims=True)
    var = ((res - mu) ** 2).mean(-1, keepdims=True)
    return (res - mu) / np.sqrt(var + LN_EPS) * ln_gamma + ln_beta


def _prep_in_maps(modality_encodings, wq, bq, wk, bk, wv, bv, wo, bo, rel_prior):
    import ml_dtypes

    s = 1.0 / math.sqrt(HD)
    wqt = np.ascontiguousarray(wq.T).astype(ml_dtypes.bfloat16)
    wkt = np.ascontiguousarray((wk * s).T).astype(ml_dtypes.bfloat16)
    wvt = np.ascontiguousarray(wv.T).astype(ml_dtypes.bfloat16)
    wot = np.ascontiguousarray(wo.T).astype(ml_dtypes.bfloat16)
    bks = bk * s
    b_eff = (bo + wo @ bv).astype(np.float32)

    bqk = np.concatenate(
        [bq.reshape(H, HD).T, bks.reshape(H, HD).T], axis=1
    ).astype(np.float32)  # [128, 16]

    # q-major prior/mask table: pmat[q_local, k_local], replicated per head so
    # the device adds it to scores for all 8 heads in one matmul
    pmat = np.full((128, 128), NEG, dtype=np.float32)
    for sm in range(SPG):
        pmat[sm * K : (sm + 1) * K, sm * K : (sm + 1) * K] = rel_prior
    pmat = np.tile(pmat, (1, H)).astype(ml_dtypes.bfloat16)  # [128, H*128]
    eye = np.eye(128, dtype=ml_dtypes.bfloat16)

    x_flat = modality_encodings.reshape(B * K, D)
    in_maps = []
    for c in range(N_CORES):
        x_core = x_flat[c * T : (c + 1) * T]
        xt = np.ascontiguousarray(x_core.T)
        in_maps.append({
            "XTB": xt.astype(ml_dtypes.bfloat16),
            "XB": x_core + b_eff,
            "WQT": wqt, "WKT": wkt, "WVT": wvt, "WOT": wot,
            "BQK": bqk, "PM": pmat, "EYE": eye,
        })
    return in_maps


def run_device(inputs, trace=False):
    """Build in_maps from full inputs, run on 8 cores, return (full_out, results)."""
    in_maps = _prep_in_maps(
        inputs["modality_encodings"], inputs["wq"], inputs["bq"], inputs["wk"],
        inputs["bk"], inputs["wv"], inputs["bv"], inputs["wo"], inputs["bo"],
        inputs["rel_prior"],
    )
    nc = _get_nc()
    res = run_bass_kernel_spmd(nc, in_maps, core_ids=list(range(N_CORES)), trace=trace)
    out = np.concatenate(
        [res.results[c]["OUT"].reshape(BC, K, D) for c in range(N_CORES)], axis=0
    )
    return out, res


def kernel(**inputs) -> np.ndarray:
    inputs = {k: np.asarray(v) for k, v in inputs.items()}
    mask = inputs["selection_mask"]
    gamma = inputs["ln_gamma"]
    beta = inputs["ln_beta"]
    if not np.all(mask > 0):
        # general-mask fallback (never hit for the spec'd inputs: fill=ones)
        return _reference_numpy(**{k: inputs[k].astype(np.float32) for k in (
            "modality_encodings", "selection_mask", "wq", "bq", "wk", "bk",
            "wv", "bv", "wo", "bo", "rel_prior", "ln_gamma", "ln_beta")}
        ).astype(np.float32)

    out, _ = run_device(inputs, trace=False)
    # device kernel skips the (identity for spec'd inputs) LN affine params
    if not (np.all(gamma == 1.0) and np.all(beta == 0.0)):
        out = out * gamma + beta
    return out.astype(np.float32)


# revision 16
# speedup vs baseline: 1.2416x; 1.0570x over previous
"""CrossModalityAttention Trainium2 kernel.

Full inputs -> full output; internally shards batch B=8192 across 8 NeuronCores
(pure data parallel). Per core: 1024 samples x K=8 modalities = 8192 tokens of
D=1024.

Device strategy (per core):
  - Host pre-transposes weights to [in,out] (lhsT layout) in bf16, folds
    1/sqrt(128) into Wk/bk, folds bv into the residual bias (attention probs
    sum to 1), passes X transposed (d-major bf16) and XB = x + bo + wo@bv
    (token-major f32 residual), plus a [128,128] prior/mask table for the
    16-samples-per-128-token-group score layout.
  - All matmuls run in bf16 (full PE rate, LDWEIGHTS half cost vs fp32);
    PSUM accumulates f32. Softmax runs q-major: scores S[q,k] land with q on
    partitions, so Z = free-dim reduce (DVE), 1/Z normalize is a
    per-partition broadcast multiply -- no cross-partition ops. Off-diagonal
    sample pairs get -30 from the prior/mask table so exp() kills them.
  - Normalized probs are PE-transposed (bf16, 1 cycle/row) to k-major and
    O^T = V^T @ P^T is produced directly in the layout the output projection
    needs -- no fp32 transposes, no per-head Z matmuls.
  - PSUM->SBUF moves run on DVE (tensor_copy), keeping the ACT engine on a
    single table set (Identity/Exp/Ln) to avoid ACT_TABLE_LOAD thrash.
  - Emission is software-pipelined: scores for group g+1 are enqueued on the
    in-order PE queue before the transpose/attend/out-proj of group g, so the
    PE stays fed while the softmax chain (DVE/ACT) of g completes.
  - rstd = exp(-0.5*ln(var+eps)) keeps every ACT function in one table set.
"""

import math

import numpy as np

import concourse.bacc as bacc
import concourse.bass as bass
import concourse.mybir as mybir
import concourse.tile as tile
from concourse.bass_utils import run_bass_kernel_spmd

N_CORES = 8
B, K, D = 8192, 8, 1024
H, HD = 8, 128
BC = B // N_CORES            # samples per core
T = BC * K                   # tokens per core (8192)
TS = 512                     # tokens per tile
NT = T // TS                 # tiles per core
GROUPS = TS // 128           # 128-token groups per tile
SPG = 128 // K               # samples per group (16)
LN_EPS = 1e-5
NEG = -30.0                  # large-negative mask for cross-sample scores

F32 = mybir.dt.float32
BF16 = mybir.dt.bfloat16

_CACHED = None  # compiled Bacc module, built once per process


def _use_single_act_table():
    """Compile with a single ACT table set: natural_log_exp_and_others holds
    exp+ln+identity+copy together, so the ACT engine never reloads tables
    between Exp (softmax), Identity (bias), Copy (PSUM moves) and Ln (LN).
    The stock act_info.json ordering makes bass pick exp_and_others for Exp
    and natural_log for Ln, which thrashes a 1.3us ACT_TABLE_LOAD twice per
    group. Both the bass-side set chooser and the walrus-side act root are
    pointed at a one-set act_info so their ids agree (id 0)."""
    import functools
    import json
    import os
    import tempfile

    from neuronxcc.driver.Job import Job
    from neuronxcc.driver.jobs.support.FindActInfo import findActInfoFile

    SET = "natural_log_exp_and_others"
    src = findActInfoFile(Job.getPackageDir(), "gen3")
    with open(src) as f:
        d = json.load(f)
    keep = [s for s in d["act_func_sets"] if s["name"] == SET]
    assert keep, f"{SET} missing from act_info.json"
    tmpdir = tempfile.mkdtemp(prefix="act_one_table_")
    srcdir = os.path.dirname(src)
    for s in keep:
        for k in d["pwp_file_keys"]:
            dst = os.path.join(tmpdir, s[k])
            if not os.path.exists(dst):
                os.symlink(os.path.join(srcdir, s[k]), dst)
    out = os.path.join(tmpdir, "act_info.json")
    with open(out, "w") as f:
        json.dump({"pwp_file_keys": d["pwp_file_keys"], "act_func_sets": keep}, f)
    os.environ["BASS_ACT_ROOT_JSON_PATH"] = out

    import concourse.hw_specs as hw_specs

    orig = hw_specs.get_activation_tables
    if getattr(orig, "_single_act_table", False):
        return

    @functools.cache
    def one_table(arch):
        full = orig(arch)
        return {SET: full[SET]}

    one_table._single_act_table = True
    hw_specs.get_activation_tables = one_table
    bacc.get_activation_tables = one_table
    try:
        import concourse.bass_interp as bass_interp

        bass_interp.get_activation_tables = one_table
    except ImportError:
        pass


def _build():
    _use_single_act_table()
    nc = bacc.Bacc("TRN2", target_bir_lowering=False, debug=False, num_devices=1)

    xtb_d = nc.dram_tensor("XTB", [D, T], BF16, kind="ExternalInput").ap()
    xb_d = nc.dram_tensor("XB", [T, D], F32, kind="ExternalInput").ap()
    wq_d = nc.dram_tensor("WQT", [D, D], BF16, kind="ExternalInput").ap()
    wk_d = nc.dram_tensor("WKT", [D, D], BF16, kind="ExternalInput").ap()
    wv_d = nc.dram_tensor("WVT", [D, D], BF16, kind="ExternalInput").ap()
    wo_d = nc.dram_tensor("WOT", [D, D], BF16, kind="ExternalInput").ap()
    bqk_d = nc.dram_tensor("BQK", [128, 2 * H], F32, kind="ExternalInput").ap()
    pm_d = nc.dram_tensor("PM", [128, H * 128], BF16, kind="ExternalInput").ap()
    eye_d = nc.dram_tensor("EYE", [128, 128], BF16, kind="ExternalInput").ap()
    out_d = nc.dram_tensor("OUT", [T, D], F32, kind="ExternalOutput").ap()

    xtb_r = xtb_d.rearrange("(c p) t -> p c t", p=128)   # [128, 8, T]

    with tile.TileContext(nc) as tc:
        with (
            tc.tile_pool(name="wpool", bufs=1) as wpool,
            tc.tile_pool(name="consts", bufs=1) as consts,
            tc.tile_pool(name="xtbp", bufs=2) as xtbp,
            tc.tile_pool(name="qkp", bufs=2) as qkp,
            tc.tile_pool(name="vp", bufs=2) as vp,
            tc.tile_pool(name="ptp", bufs=2) as ptp,
            tc.tile_pool(name="pnp", bufs=3) as pnp,
            tc.tile_pool(name="ptsp", bufs=2) as ptsp,
            tc.tile_pool(name="otsp", bufs=2) as otsp,
            tc.tile_pool(name="xbp", bufs=2) as xbp,
            tc.tile_pool(name="smalls", bufs=4) as smalls,
            tc.tile_pool(name="projps", bufs=2, space="PSUM") as projps,
            tc.tile_pool(name="attps", bufs=2, space="PSUM") as attps,
            tc.tile_pool(name="ptTp", bufs=2, space="PSUM") as ptTp,
        ):
            # ---- constants / weights (resident) ----
            wq = wpool.tile([128, 8, D], BF16, tag="w_q")
            nc.sync.dma_start(wq[:], wq_d.rearrange("(c p) m -> p c m", p=128))
            wk = wpool.tile([128, 8, D], BF16, tag="w_k")
            nc.sync.dma_start(wk[:], wk_d.rearrange("(c p) m -> p c m", p=128))
            wv = wpool.tile([128, 8, D], BF16, tag="w_v")
            nc.sync.dma_start(wv[:], wv_d.rearrange("(c p) m -> p c m", p=128))
            wo = wpool.tile([128, 8, D], BF16, tag="w_o")
            nc.sync.dma_start(wo[:], wo_d.rearrange("(c p) m -> p c m", p=128))
            bqk = consts.tile([128, 2 * H], F32)
            nc.sync.dma_start(bqk[:], bqk_d)
            pm = consts.tile([128, H * 128], BF16)
            nc.sync.dma_start(pm[:], pm_d)
            eye = consts.tile([128, 128], BF16)
            nc.sync.dma_start(eye[:], eye_d)
            eps = consts.tile([128, 1], F32)
            nc.vector.memset(eps[:], LN_EPS)

            pending = []  # deferred per-group state, depth-2 software pipeline

            def finish(st8):
                """Transpose probs, attend, out-proj, residual+LN for a group
                whose scores/softmax chain was already emitted."""
                pn, v, g, tok0 = st8
                ptT = ptTp.tile([128, H, 128], BF16)
                for h in range(H):
                    nc.tensor.transpose(ptT[:, h, :], pn[:, h, :], eye[:])
                pts = ptsp.tile([128, H, 128], BF16)
                nc.scalar.activation(
                    pts[:], ptT[:], mybir.ActivationFunctionType.Copy
                )
                ot = attps.tile([128, H, 128], F32, tag="attps")
                for h in range(H):
                    nc.tensor.matmul(ot[:, h, :], v[:, g, h, :], pts[:, h, :])
                ots = otsp.tile([128, H, 128], BF16)
                nc.scalar.activation(
                    ots[:], ot[:], mybir.ActivationFunctionType.Copy
                )

                xb = xbp.tile([128, D], F32)
                nc.sync.dma_start(
                    xb[:], xb_d[tok0 + g * 128 : tok0 + (g + 1) * 128, :]
                )
                for half in range(2):
                    yp = projps.tile([128, 512], F32, tag="projps")
                    for c in range(8):
                        nc.tensor.matmul(
                            yp[:],
                            ots[:, c, :],
                            wo[:, c, half * 512 : (half + 1) * 512],
                            start=(c == 0),
                            stop=(c == 7),
                        )
                    nc.vector.tensor_tensor(
                        xb[:, half * 512 : (half + 1) * 512],
                        xb[:, half * 512 : (half + 1) * 512],
                        yp[:],
                        mybir.AluOpType.add,
                    )
                stats = smalls.tile([128, 2, 6], F32, tag="stats")
                for sg in range(2):
                    nc.vector.bn_stats(
                        stats[:, sg, :], xb[:, sg * 512 : (sg + 1) * 512]
                    )
                mv = smalls.tile([128, 2], F32, tag="mv")
                nc.vector.bn_aggr(mv[:], stats[:])
                # rstd = exp(-0.5*ln(var+eps)); ln+exp live in one ACT
                # table set (sqrt does not), avoiding table reloads
                sd = smalls.tile([128, 1], F32, tag="sd")
                nc.scalar.activation(
                    sd[:],
                    mv[:, 1:2],
                    mybir.ActivationFunctionType.Ln,
                    bias=eps[:],
                )
                nc.scalar.activation(
                    sd[:], sd[:], mybir.ActivationFunctionType.Exp, scale=-0.5
                )
                nc.vector.tensor_scalar(
                    out=xb[:],
                    in0=xb[:],
                    scalar1=mv[:, 0:1],
                    scalar2=sd[:],
                    op0=mybir.AluOpType.subtract,
                    op1=mybir.AluOpType.mult,
                )
                nc.sync.dma_start(
                    out_d[tok0 + g * 128 : tok0 + (g + 1) * 128, :], xb[:]
                )

            # xtb is prefetched one tile ahead so tile t's Q/K matmuls never
            # wait on the DMA
            xtb_tiles = {}

            def load_xtb(t):
                xtb_tiles[t] = xtbp.tile([128, 8, TS], BF16, name="xtb", tag="xtb")
                nc.sync.dma_start(
                    xtb_tiles[t][:], xtb_r[:, :, t * TS : (t + 1) * TS]
                )

            load_xtb(0)
            for t in range(NT):
                tok0 = t * TS
                if t + 1 < NT:
                    load_xtb(t + 1)
                xtb = xtb_tiles.pop(t)

                # ---- Q^T, K^T projections (bf16): [d_head(128) x tok(TS)]
                qt = qkp.tile([128, H, TS], BF16, tag="qt")
                kt = qkp.tile([128, H, TS], BF16, tag="kt")
                for wt, dst, bias_col0 in ((wq, qt, 0), (wk, kt, H)):
                    for h in range(H):
                        ps = projps.tile([128, TS], F32, tag="projps")
                        for c in range(8):
                            nc.tensor.matmul(
                                ps[:],
                                wt[:, c, h * HD : (h + 1) * HD],
                                xtb[:, c, :],
                                start=(c == 0),
                                stop=(c == 7),
                            )
                        nc.scalar.activation(
                            dst[:, h, :],
                            ps[:],
                            mybir.ActivationFunctionType.Identity,
                            bias=bqk[:, bias_col0 + h : bias_col0 + h + 1],
                        )

                # ---- V projection (bf16), token-major
                v = vp.tile([128, GROUPS, H, HD], BF16, tag="v")
                for sub in range(GROUPS):
                    for half in range(2):
                        psv = projps.tile([128, 512], F32, tag="projps")
                        for c in range(8):
                            nc.tensor.matmul(
                                psv[:],
                                xtb[:, c, sub * 128 : (sub + 1) * 128],
                                wv[:, c, half * 512 : (half + 1) * 512],
                                start=(c == 0),
                                stop=(c == 7),
                            )
                        nc.vector.tensor_copy(
                            v[:, sub, 4 * half : 4 * half + 4, :],
                            psv.rearrange("p (a b) -> p a b", a=4),
                        )

                # ---- attention per 128-token group, q-major softmax.
                # Scores for group g are emitted, then the deferred
                # transpose/attend/out-proj of the previous group, so the PE
                # queue stays busy while g's softmax chain runs on DVE/ACT.
                for g in range(GROUPS):
                    gsl = slice(g * 128, (g + 1) * 128)
                    st = attps.tile([128, H, 128], F32, tag="attps")
                    # prior/mask lands first via PSUM accumulation:
                    # st[q, (h,k)] = eye^T @ pm, then += Q^T K per head
                    # (two matmuls: a [128,1024] f32 dst would span 2 banks)
                    for hh in range(2):
                        nc.tensor.matmul(
                            st[:, 4 * hh : 4 * hh + 4, :].rearrange(
                                "p a b -> p (a b)"
                            ),
                            eye[:],
                            pm[:, 4 * hh * 128 : (4 * hh + 4) * 128],
                            start=True,
                            stop=False,
                            skip_group_check=True,
                        )
                    for h in range(H):
                        # S[q, k] += sum_d Q^T[d, q] K^T[d, k]
                        nc.tensor.matmul(
                            st[:, h, :],
                            qt[:, h, gsl],
                            kt[:, h, gsl],
                            start=False,
                            stop=True,
                            skip_group_check=True,
                        )
                    pt = ptp.tile([128, H, 128], BF16)
                    nc.scalar.activation(
                        pt[:], st[:], mybir.ActivationFunctionType.Exp
                    )
                    z = smalls.tile([128, H], F32, tag="z")
                    nc.vector.tensor_reduce(
                        z[:], pt[:], mybir.AxisListType.X, mybir.AluOpType.add
                    )
                    rz = smalls.tile([128, H], F32, tag="rz")
                    nc.vector.reciprocal(rz[:], z[:])
                    pn = pnp.tile([128, H, 128], BF16)
                    nc.vector.tensor_tensor(
                        pn[:],
                        pt[:],
                        rz[:, :, None].to_broadcast((128, H, 128)),
                        mybir.AluOpType.mult,
                    )
                    pending.append((pn, v, g, tok0))
                    if len(pending) > 2:
                        finish(pending.pop(0))

            for p in pending:
                finish(p)

    nc.compile()
    return nc


def _get_nc():
    global _CACHED
    if _CACHED is None:
        _CACHED = _build()
    return _CACHED


def _reference_numpy(modality_encodings, selection_mask, wq, bq, wk, bk, wv, bv,
                     wo, bo, rel_prior, ln_gamma, ln_beta):
    """Slow fallback, exact port of the reference (used only if inputs fall
    outside the fast path's assumptions: non-trivial mask)."""
    x = modality_encodings.astype(np.float32)
    Bn, Kn, Dn = x.shape
    Hd = Dn // H
    q = (x @ wq.T + bq).reshape(Bn, Kn, H, Hd).transpose(0, 2, 1, 3)
    k = (x @ wk.T + bk).reshape(Bn, Kn, H, Hd).transpose(0, 2, 1, 3)
    v = (x @ wv.T + bv).reshape(Bn, Kn, H, Hd).transpose(0, 2, 1, 3)
    scores = np.einsum("bhqd,bhkd->bhqk", q, k) / math.sqrt(Hd)
    scores = scores + rel_prior[None, None]
    mask2d = (selection_mask[:, :, None] * selection_mask[:, None, :]) > 0
    scores = np.where(mask2d[:, None], scores, -np.inf)
    scores = scores - scores.max(axis=-1, keepdims=True)
    e = np.exp(scores)
    attn = e / e.sum(axis=-1, keepdims=True)
    out = np.einsum("bhqk,bhkd->bhqd", attn, v)
    out = out.transpose(0, 2, 1, 3).reshape(Bn, Kn, Dn)
    out = out @ wo.T + bo
    res = x + out
    mu = res.mean(-1, keepdims=True)
    var = ((res - mu) ** 2).mean(-1, keepdims=True)
    return (res - mu) / np.sqrt(var + LN_EPS) * ln_gamma + ln_beta


def _prep_in_maps(modality_encodings, wq, bq, wk, bk, wv, bv, wo, bo, rel_prior):
    import ml_dtypes

    s = 1.0 / math.sqrt(HD)
    wqt = np.ascontiguousarray(wq.T).astype(ml_dtypes.bfloat16)
    wkt = np.ascontiguousarray((wk * s).T).astype(ml_dtypes.bfloat16)
    wvt = np.ascontiguousarray(wv.T).astype(ml_dtypes.bfloat16)
    wot = np.ascontiguousarray(wo.T).astype(ml_dtypes.bfloat16)
    bks = bk * s
    b_eff = (bo + wo @ bv).astype(np.float32)

    bqk = np.concatenate(
        [bq.reshape(H, HD).T, bks.reshape(H, HD).T], axis=1
    ).astype(np.float32)  # [128, 16]

    # q-major prior/mask table: pmat[q_local, k_local], replicated per head so
    # the device adds it to scores for all 8 heads in one matmul
    pmat = np.full((128, 128), NEG, dtype=np.float32)
    for sm in range(SPG):
        pmat[sm * K : (sm + 1) * K, sm * K : (sm + 1) * K] = rel_prior
    pmat = np.tile(pmat, (1, H)).astype(ml_dtypes.bfloat16)  # [128, H*128]
    eye = np.eye(128, dtype=ml_dtypes.bfloat16)

    x_flat = modality_encodings.reshape(B * K, D)
    in_maps = []
    for c in range(N_CORES):
        x_core = x_flat[c * T : (c + 1) * T]
        xt = np.ascontiguousarray(x_core.T)
        in_maps.append({
            "XTB": xt.astype(ml_dtypes.bfloat16),
            "XB": x_core + b_eff,
            "WQT": wqt, "WKT": wkt, "WVT": wvt, "WOT": wot,
            "BQK": bqk, "PM": pmat, "EYE": eye,
        })
    return in_maps


def run_device(inputs, trace=False):
    """Build in_maps from full inputs, run on 8 cores, return (full_out, results)."""
    in_maps = _prep_in_maps(
        inputs["modality_encodings"], inputs["wq"], inputs["bq"], inputs["wk"],
        inputs["bk"], inputs["wv"], inputs["bv"], inputs["wo"], inputs["bo"],
        inputs["rel_prior"],
    )
    nc = _get_nc()
    res = run_bass_kernel_spmd(nc, in_maps, core_ids=list(range(N_CORES)), trace=trace)
    out = np.concatenate(
        [res.results[c]["OUT"].reshape(BC, K, D) for c in range(N_CORES)], axis=0
    )
    return out, res


def kernel(**inputs) -> np.ndarray:
    inputs = {k: np.asarray(v) for k, v in inputs.items()}
    mask = inputs["selection_mask"]
    gamma = inputs["ln_gamma"]
    beta = inputs["ln_beta"]
    if not np.all(mask > 0):
        # general-mask fallback (never hit for the spec'd inputs: fill=ones)
        return _reference_numpy(**{k: inputs[k].astype(np.float32) for k in (
            "modality_encodings", "selection_mask", "wq", "bq", "wk", "bk",
            "wv", "bv", "wo", "bo", "rel_prior", "ln_gamma", "ln_beta")}
        ).astype(np.float32)

    out, _ = run_device(inputs, trace=False)
    # device kernel skips the (identity for spec'd inputs) LN affine params
    if not (np.all(gamma == 1.0) and np.all(beta == 0.0)):
        out = out * gamma + beta
    return out.astype(np.float32)
